# revision 5
# baseline (speedup 1.0000x reference)
"""Trainium2 Bass kernel for nn_MathReasoningModule.

Computes, for num_steps iterations from h=x:
    ops[t]   = gelu(h @ w_op1 + b_op1) @ w_op2 + b_op2
    nums[t]  = gelu(h @ w_num1 + b_num1) @ w_num2 + b_num2
    h        = GRUCell(input=h, hidden=h)       (states[t] = pre-update h)
returns (final_state, ops, nums, states) like the reference.

Strategy (per core, batch sharded 8 ways, 2048 rows/core):
  - Feature-major on-chip layout [feat_in_chunk(128 part), k, batch(free)].
  - One fused weight matrix [H, 6H]: r/z gates use (w_ih+w_hh) combined
    (input==hidden), so per-step matmul width is 6H not 8H.
  - bf16 matmul inputs, fp32 PSUM accumulate, fp32 gates/state.
  - GELU via erf (exact): keeps sigmoid/tanh/erf in one ACT table set.
  - h round-trips DRAM fp32 between steps (SBUF holds weights-stream +
    bf16 activations + small streaming pools only).
  - states output (batch-major) via PE transpose of h_new tiles.
"""

import sys
import numpy as np

try:
    import concourse.bass as bass
except ImportError:  # fresh dir without sitecustomize path
    sys.path.insert(0, "/opt/trn_rl_repo")
    import concourse.bass as bass
import concourse.tile as tile
import ml_dtypes
from concourse import mybir
from concourse.bass_utils import run_bass_kernel_spmd
from concourse.masks import make_identity

F32 = mybir.dt.float32
BF16 = mybir.dt.bfloat16
ADD = mybir.AluOpType.add
MULT = mybir.AluOpType.mult
AF = mybir.ActivationFunctionType

H = 1024
B_FULL = 16384
N_CORES = 8
BC = B_FULL // N_CORES  # 2048 batch rows per core
KC = H // 128           # 8 contraction chunks
NB = 4                  # batch sub-blocks per core
BS = BC // NB           # 512 (one PSUM bank of fp32)
NFO = 6 * KC            # 48 fused output tiles of 128 features
INV_SQRT2 = 0.7071067811865476


def _split_excess_waits(nc, max_per=1):
    """walrus in this container accepts only one sync-wait condition per
    instruction; move excess on_wait conditions onto preceding nops."""
    n_split = 0
    for fn in nc.m.functions:
        for blk in fn.blocks:
            new_list = []
            for inst in blk.instructions:
                si = inst.sync_info
                if si is not None and si.on_wait is not None and len(si.on_wait) > max_per:
                    waits = list(si.on_wait)
                    keep = waits[-max_per:]
                    excess = waits[:-max_per]
                    for i in range(0, len(excess), max_per):
                        g = excess[i:i + max_per]
                        nop = mybir.InstNoOp(name=f"{inst.name}-ws{n_split}", ins=[], outs=[])
                        n_split += 1
                        nop.engine = inst.engine
                        nop.sync_info = mybir.SyncInfo(on_wait=g, on_update=[])
                        nop.bass_nofuse = True
                        new_list.append(nop)
                        nc.register_instruction(nop, overwrite=True)
                    inst.sync_info = mybir.SyncInfo(on_wait=keep, on_update=list(si.on_update or []))
                new_list.append(inst)
            blk.instructions = new_list
    return n_split


def build_nc(num_steps, n_repeat=1):
    S = int(num_steps)
    nc = bass.Bass("TRN2", target_bir_lowering=False)

    h0 = nc.dram_tensor("h0", [128, KC, BC], F32, kind="ExternalInput")
    wbig = nc.dram_tensor("wbig", [128, NFO, H], BF16, kind="ExternalInput")
    w2_in = nc.dram_tensor("w2", [128, KC, 6], BF16, kind="ExternalInput")
    biases_in = nc.dram_tensor("biases", [128, NFO], F32, kind="ExternalInput")
    berf_in = nc.dram_tensor("berf", [128, 2 * KC], F32, kind="ExternalInput")
    b2_in = nc.dram_tensor("b2", [128, 2], F32, kind="ExternalInput")

    states_o = nc.dram_tensor("states_o", [max(S - 1, 1), BC // 128, 128, H], F32, kind="ExternalOutput")
    final_o = nc.dram_tensor("final_o", [BC // 128, 128, H], F32, kind="ExternalOutput")
    ops_o = nc.dram_tensor("ops_o", [S, 5, BC], F32, kind="ExternalOutput")
    nums_o = nc.dram_tensor("nums_o", [S, 1, BC], F32, kind="ExternalOutput")

    def bsl(b):
        return slice(b * BS, (b + 1) * BS)

    from contextlib import ExitStack
    with tile.TileContext(nc) as tc:
        with ExitStack() as ctx:
            pool = lambda name, bufs, **kw: ctx.enter_context(tc.tile_pool(name=name, bufs=bufs, **kw))
            dram = pool("dram", 1, space="DRAM")
            singles = pool("singles", 1)
            hbf_pool = pool("hbf", 2)
            w_pool = pool("wstream", 3)
            h32_pool = pool("h32", 3)
            u_pool = pool("upool", 3)
            e_pool = pool("epool", 3)
            g_pool = pool("gpool", 4)
            r_pool = pool("rpool", 5)
            z_pool = pool("zpool", 5)
            t_pool = pool("tpool", 4)
            t2_pool = pool("t2pool", 3)
            n_pool = pool("npool", 3)
            d_pool = pool("dpool", 3)
            e2_pool = pool("e2pool", 3)
            hnew_pool = pool("hnew", 4)
            tl_pool = pool("tload", 2)
            tstage_pool = pool("tstage", 2)
            gh_pool = pool("ghead", 2)
            headst_pool = pool("headst", 2)
            psum_big = pool("psA", 4, space="PSUM")
            psum_T = pool("psT", 2, space="PSUM")
            psum_head = pool("psH", 2, space="PSUM")
            hA = dram.tile([128, KC, BC], F32, tag="hA")
            hB = dram.tile([128, KC, BC], F32, tag="hB")
            g1d = dram.tile([128, KC, BC], BF16, tag="g1d")
            g2d = dram.tile([128, KC, BC], BF16, tag="g2d")

            w2_sb = singles.tile([128, KC, 6], BF16, tag="w2sb")
            nc.sync.dma_start(out=w2_sb, in_=w2_in[:])
            biases = singles.tile([128, NFO], F32, tag="biases")
            nc.sync.dma_start(out=biases, in_=biases_in[:])
            berf = singles.tile([128, 2 * KC], F32, tag="berf")
            nc.sync.dma_start(out=berf, in_=berf_in[:])
            b2 = singles.tile([128, 2], F32, tag="b2")
            nc.sync.dma_start(out=b2, in_=b2_in[:])
            ident = singles.tile([128, 128], F32, tag="ident")
            make_identity(nc, ident[:])

            def transpose_phase(src, dst):
                # src: [128, KC, BC] f32 feature-major -> dst [BC//128, 128, H] batch-major
                for chunk in range(BC // 128):
                    tl = tl_pool.tile([128, KC, 128], F32, tag="tl")
                    nc.sync.dma_start(out=tl, in_=src[:, :, chunk * 128:(chunk + 1) * 128])
                    st = tstage_pool.tile([128, H], F32, tag="st")
                    for half in range(KC // 4):
                        pT = psum_T.tile([128, 4, 128], F32, tag="pT")
                        for jj in range(4):
                            j = half * 4 + jj
                            nc.tensor.transpose(out=pT[:, jj, :], in_=tl[:, j, :], identity=ident[:])
                        nc.vector.tensor_copy(out=st[:, half * 512:(half + 1) * 512], in_=pT[:])
                    nc.sync.dma_start(out=dst[chunk], in_=st)

            for _rep in range(n_repeat):
              for t in range(S):
                rb = h0 if t == 0 else (hA if t % 2 == 1 else hB)
                wb = hA if t % 2 == 0 else hB

                # bf16 cast of this step's h (SWDGE dma cast)
                h_bf = hbf_pool.tile([128, KC, BC], BF16, tag="hbf")
                for b in range(NB):
                    nc.gpsimd.dma_start(out=h_bf[:, :, bsl(b)], in_=rb[:, :, bsl(b)])

                r_by_b = {}
                z_by_b = {}
                t_by_b = {}
                for j in range(KC):
                    for kind in range(6):  # r, z, hn, in, op1, num1
                        fo = j * 6 + kind
                        w_t = w_pool.tile([128, KC, 128], BF16, tag="wt")
                        nc.sync.dma_start(out=w_t, in_=wbig[:, fo, :])
                        for b in range(NB):
                            ps = psum_big.tile([128, BS], F32, tag="ps")
                            for k in range(KC):
                                nc.tensor.matmul(
                                    ps[:], lhsT=w_t[:, k, :], rhs=h_bf[:, k, bsl(b)],
                                    start=(k == 0), stop=(k == KC - 1),
                                )
                            bcol = biases[:, fo:fo + 1]
                            if kind == 0:
                                r_ = r_pool.tile([128, BS], F32, tag="r")
                                nc.scalar.activation(out=r_[:], in_=ps[:], func=AF.Sigmoid, bias=bcol, scale=1.0)
                                r_by_b[b] = r_
                            elif kind == 1:
                                z_ = z_pool.tile([128, BS], F32, tag="z")
                                nc.scalar.activation(out=z_[:], in_=ps[:], func=AF.Sigmoid, bias=bcol, scale=1.0)
                                z_by_b[b] = z_
                            elif kind == 2:
                                t_ = t_pool.tile([128, BS], F32, tag="t")
                                nc.vector.scalar_tensor_tensor(
                                    out=t_[:], in0=ps[:], scalar=bcol, in1=r_by_b[b][:],
                                    op0=ADD, op1=MULT,
                                )
                                t_by_b[b] = t_
                            elif kind == 3:
                                t2 = t2_pool.tile([128, BS], F32, tag="t2")
                                nc.vector.tensor_add(out=t2[:], in0=ps[:], in1=t_by_b[b][:])
                                n_ = n_pool.tile([128, BS], F32, tag="n")
                                nc.scalar.activation(out=n_[:], in_=t2[:], func=AF.Tanh, bias=bcol, scale=1.0)
                                h32 = h32_pool.tile([128, BS], F32, tag="h32")
                                nc.sync.dma_start(out=h32, in_=rb[:, j, bsl(b)])
                                d_ = d_pool.tile([128, BS], F32, tag="d")
                                nc.vector.tensor_sub(out=d_[:], in0=h32[:], in1=n_[:])
                                e2 = e2_pool.tile([128, BS], F32, tag="e2")
                                nc.vector.tensor_mul(out=e2[:], in0=z_by_b[b][:], in1=d_[:])
                                hn_ = hnew_pool.tile([128, BS], F32, tag="hnw")
                                nc.vector.tensor_add(out=hn_[:], in0=e2[:], in1=n_[:])
                                nc.sync.dma_start(out=wb[:, j, bsl(b)], in_=hn_)
                            else:
                                # op1 (kind 4) -> g1d, num1 (kind 5) -> g2d
                                u_ = u_pool.tile([128, BS], F32, tag="u")
                                nc.vector.tensor_scalar_add(out=u_[:], in0=ps[:], scalar1=bcol)
                                e_ = e_pool.tile([128, BS], F32, tag="e")
                                ecol = berf[:, (0 if kind == 4 else KC) + j:(0 if kind == 4 else KC) + j + 1]
                                nc.scalar.activation(out=e_[:], in_=ps[:], func=AF.Erf, bias=ecol, scale=INV_SQRT2)
                                g_ = g_pool.tile([128, BS], BF16, tag="g")
                                # 2*gelu(u) = u*(1+erf(u/sqrt2)); the 0.5 is folded into w2
                                nc.vector.scalar_tensor_tensor(
                                    out=g_[:], in0=e_[:], scalar=1.0, in1=u_[:], op0=ADD, op1=MULT,
                                )
                                gd = g1d if kind == 4 else g2d
                                nc.sync.dma_start(out=gd[:, j, bsl(b)], in_=g_)

                # states[t] = pre-update h = h_{t+1} written this step? No:
                # states[t] is THIS step's input h; host supplies states[0]=x,
                # so transpose rb (t>=1) into states_o[t-1].
                if t >= 1:
                    transpose_phase(rb, states_o[t - 1])

                # head matmuls from DRAM-staged gelu activations
                for b in range(NB):
                    gh1 = gh_pool.tile([128, KC, BS], BF16, tag="gh")
                    nc.sync.dma_start(out=gh1, in_=g1d[:, :, bsl(b)])
                    po = psum_head.tile([5, BS], F32, tag="ph")
                    for k in range(KC):
                        nc.tensor.matmul(po[:], lhsT=w2_sb[:, k, 0:5], rhs=gh1[:, k, :],
                                         start=(k == 0), stop=(k == KC - 1))
                    so = headst_pool.tile([5, BS], F32, tag="hso")
                    nc.vector.tensor_scalar_add(out=so[:], in0=po[:], scalar1=b2[0:5, 0:1])
                    nc.sync.dma_start(out=ops_o[t, :, bsl(b)], in_=so)

                    gh2 = gh_pool.tile([128, KC, BS], BF16, tag="gh")
                    nc.sync.dma_start(out=gh2, in_=g2d[:, :, bsl(b)])
                    pn = psum_head.tile([1, BS], F32, tag="ph")
                    for k in range(KC):
                        nc.tensor.matmul(pn[:], lhsT=w2_sb[:, k, 5:6], rhs=gh2[:, k, :],
                                         start=(k == 0), stop=(k == KC - 1))
                    sn = headst_pool.tile([1, BS], F32, tag="hsn")
                    nc.vector.tensor_scalar_add(out=sn[:], in0=pn[:], scalar1=b2[0:1, 1:2])
                    nc.sync.dma_start(out=nums_o[t, :, bsl(b)], in_=sn)

            # final_state = h_{S} = wb of last step
            final_src = hA if (S - 1) % 2 == 0 else hB
            transpose_phase(final_src, final_o)

    _split_excess_waits(nc, max_per=1)
    return nc


_NC_CACHE = {}


def _get_nc(num_steps):
    if num_steps not in _NC_CACHE:
        _NC_CACHE[num_steps] = build_nc(num_steps)
    return _NC_CACHE[num_steps]


def _prep_weights(w_op1, b_op1, w_op2, b_op2, w_num1, b_num1, w_num2, b_num2,
                  w_ih, b_ih, w_hh, b_hh):
    f = np.float32
    w_ih = np.asarray(w_ih, f); w_hh = np.asarray(w_hh, f)
    b_ih = np.asarray(b_ih, f); b_hh = np.asarray(b_hh, f)
    Wr = (w_ih[0:H] + w_hh[0:H]).T.astype(f)          # [H, H]
    Wz = (w_ih[H:2 * H] + w_hh[H:2 * H]).T.astype(f)
    Whn = w_hh[2 * H:].T.astype(f)
    Win = w_ih[2 * H:].T.astype(f)
    br = (b_ih[0:H] + b_hh[0:H]).astype(f)
    bz = (b_ih[H:2 * H] + b_hh[H:2 * H]).astype(f)
    bhn = b_hh[2 * H:].astype(f)
    bin_ = b_ih[2 * H:].astype(f)
    w_op1 = np.asarray(w_op1, f); w_num1 = np.asarray(w_num1, f)
    b_op1 = np.asarray(b_op1, f); b_num1 = np.asarray(b_num1, f)

    # fused weight: per j-group of 128 output features: [r, z, hn, in, op1, num1]
    Wcat = np.empty((H, NFO * 128), f)
    bias_cols = np.empty((128, NFO), f)
    kinds = [(Wr, br), (Wz, bz), (Whn, bhn), (Win, bin_), (w_op1, b_op1), (w_num1, b_num1)]
    for j in range(KC):
        for kind, (Wk, bk) in enumerate(kinds):
            fo = j * 6 + kind
            Wcat[:, fo * 128:(fo + 1) * 128] = Wk[:, j * 128:(j + 1) * 128]
            bias_cols[:, fo] = bk[j * 128:(j + 1) * 128]

    # wbig[p, fo, k*128+m] = Wcat[k*128+p, fo*128+m]
    wbig = np.ascontiguousarray(
        Wcat.reshape(KC, 128, NFO, 128).transpose(1, 2, 0, 3).reshape(128, NFO, H)
    ).astype(ml_dtypes.bfloat16)

    berf = np.empty((128, 2 * KC), f)
    for j in range(KC):
        berf[:, j] = b_op1[j * 128:(j + 1) * 128] * INV_SQRT2
        berf[:, KC + j] = b_num1[j * 128:(j + 1) * 128] * INV_SQRT2

    w2 = np.zeros((128, KC, 6), f)
    w2[:, :, 0:5] = 0.5 * np.asarray(w_op2, f).reshape(KC, 128, 5).transpose(1, 0, 2)
    w2[:, :, 5] = 0.5 * np.asarray(w_num2, f).reshape(KC, 128).transpose(1, 0)
    w2 = w2.astype(ml_dtypes.bfloat16)

    b2 = np.zeros((128, 2), f)
    b2[0:5, 0] = np.asarray(b_op2, f)
    b2[0:1, 1] = np.asarray(b_num2, f)
    return wbig, bias_cols, berf, w2, b2


def kernel(x, num_steps, w_op1, b_op1, w_op2, b_op2,
           w_num1, b_num1, w_num2, b_num2,
           w_ih, b_ih, w_hh, b_hh):
    S = int(num_steps)
    x = np.asarray(x, np.float32)
    assert x.shape == (B_FULL, H)

    wbig, bias_cols, berf, w2, b2 = _prep_weights(
        w_op1, b_op1, w_op2, b_op2, w_num1, b_num1, w_num2, b_num2,
        w_ih, b_ih, w_hh, b_hh)

    nc = _get_nc(S)
    in_maps = []
    for c in range(N_CORES):
        xs = x[c * BC:(c + 1) * BC]                       # [BC, H]
        h0 = np.ascontiguousarray(
            xs.T.reshape(KC, 128, BC).transpose(1, 0, 2))  # [128, KC, BC]
        in_maps.append({
            "h0": h0, "wbig": wbig, "w2": w2,
            "biases": bias_cols, "berf": berf, "b2": b2,
        })

    res = run_bass_kernel_spmd(nc, in_maps, list(range(N_CORES)))
    results = res.results

    final = np.concatenate([r["final_o"].reshape(BC, H) for r in results], axis=0)
    states = np.empty((S, B_FULL, H), np.float32)
    states[0] = x
    for t in range(1, S):
        states[t] = np.concatenate(
            [r["states_o"][t - 1].reshape(BC, H) for r in results], axis=0)
    ops = np.concatenate([r["ops_o"].transpose(0, 2, 1) for r in results], axis=1)
    nums = np.concatenate([r["nums_o"].transpose(0, 2, 1) for r in results], axis=1)
    return final, ops, nums, states


# revision 9
# speedup vs baseline: 1.3740x; 1.3740x over previous
"""Trainium2 Bass kernel for nn_MathReasoningModule.

Computes, for num_steps iterations from h=x:
    ops[t]   = gelu(h @ w_op1 + b_op1) @ w_op2 + b_op2
    nums[t]  = gelu(h @ w_num1 + b_num1) @ w_num2 + b_num2
    h        = GRUCell(input=h, hidden=h)       (states[t] = pre-update h)
returns (final_state, ops, nums, states) like the reference.

Strategy (per core, batch sharded 8 ways, 2048 rows/core):
  - Feature-major on-chip layout [feat_in_chunk(128 part), k, batch(free)].
  - One fused weight matrix [H, 6H]: r/z gates use (w_ih+w_hh) combined
    (input==hidden), so per-step matmul width is 6H not 8H.
  - bf16 matmul inputs, fp32 PSUM accumulate, fp32 gates/state.
  - GELU via erf (exact): keeps sigmoid/tanh/erf in one ACT table set.
  - h round-trips DRAM fp32 between steps (SBUF holds weights-stream +
    bf16 activations + small streaming pools only).
  - states output (batch-major) via PE transpose of h_new tiles.
"""

import sys
import numpy as np

try:
    import concourse.bass as bass
except ImportError:  # fresh dir without sitecustomize path
    sys.path.insert(0, "/opt/trn_rl_repo")
    import concourse.bass as bass
import concourse.tile as tile
import ml_dtypes
from concourse import mybir
from concourse.bass_utils import run_bass_kernel_spmd
from concourse.masks import make_identity

F32 = mybir.dt.float32
BF16 = mybir.dt.bfloat16
ADD = mybir.AluOpType.add
MULT = mybir.AluOpType.mult
AF = mybir.ActivationFunctionType

H = 1024
B_FULL = 16384
N_CORES = 8
BC = B_FULL // N_CORES  # 2048 batch rows per core
KC = H // 128           # 8 contraction chunks
NB = 4                  # batch sub-blocks per core
BS = BC // NB           # 512 (one PSUM bank of fp32)
NFO = 6 * KC            # 48 fused output tiles of 128 features
INV_SQRT2 = 0.7071067811865476


def _split_excess_waits(nc, max_per=1):
    """walrus in this container accepts only one sync-wait condition per
    instruction; move excess on_wait conditions onto preceding nops."""
    n_split = 0
    for fn in nc.m.functions:
        for blk in fn.blocks:
            new_list = []
            for inst in blk.instructions:
                si = inst.sync_info
                if si is not None and si.on_wait is not None and len(si.on_wait) > max_per:
                    waits = list(si.on_wait)
                    keep = waits[-max_per:]
                    excess = waits[:-max_per]
                    for i in range(0, len(excess), max_per):
                        g = excess[i:i + max_per]
                        nop = mybir.InstNoOp(name=f"{inst.name}-ws{n_split}", ins=[], outs=[])
                        n_split += 1
                        nop.engine = inst.engine
                        nop.sync_info = mybir.SyncInfo(on_wait=g, on_update=[])
                        nop.bass_nofuse = True
                        new_list.append(nop)
                        nc.register_instruction(nop, overwrite=True)
                    inst.sync_info = mybir.SyncInfo(on_wait=keep, on_update=list(si.on_update or []))
                new_list.append(inst)
            blk.instructions = new_list
    return n_split


def build_nc(num_steps, n_repeat=1):
    S = int(num_steps)
    nc = bass.Bass("TRN2", target_bir_lowering=False)

    h0 = nc.dram_tensor("h0", [128, KC, BC], F32, kind="ExternalInput")
    wbig = nc.dram_tensor("wbig", [128, NFO, H], BF16, kind="ExternalInput")
    w2_in = nc.dram_tensor("w2", [128, KC, 6], BF16, kind="ExternalInput")
    biases_in = nc.dram_tensor("biases", [128, NFO], F32, kind="ExternalInput")
    berf_in = nc.dram_tensor("berf", [128, 2 * KC], F32, kind="ExternalInput")
    b2_in = nc.dram_tensor("b2", [128, 2], F32, kind="ExternalInput")

    states_o = nc.dram_tensor("states_o", [max(S - 1, 1), BC // 128, 128, H], F32, kind="ExternalOutput")
    final_o = nc.dram_tensor("final_o", [BC // 128, 128, H], F32, kind="ExternalOutput")
    ops_o = nc.dram_tensor("ops_o", [S, 5, BC], F32, kind="ExternalOutput")
    nums_o = nc.dram_tensor("nums_o", [S, 1, BC], F32, kind="ExternalOutput")

    def bsl(b):
        return slice(b * BS, (b + 1) * BS)

    from contextlib import ExitStack
    with tile.TileContext(nc) as tc:
        with ExitStack() as ctx:
            pool = lambda name, bufs, **kw: ctx.enter_context(tc.tile_pool(name=name, bufs=bufs, **kw))
            dram = pool("dram", 1, space="DRAM")
            singles = pool("singles", 1)
            hbf_pool = pool("hbf", 2)
            w_pool = pool("wstream", 3)
            h32_pool = pool("h32", 3)
            u_pool = pool("upool", 3)
            e_pool = pool("epool", 3)
            g_pool = pool("gpool", 4)
            r_pool = pool("rpool", 5)
            z_pool = pool("zpool", 5)
            t_pool = pool("tpool", 4)
            t2_pool = pool("t2pool", 3)
            n_pool = pool("npool", 3)
            d_pool = pool("dpool", 3)
            e2_pool = pool("e2pool", 3)
            hnew_pool = pool("hnew", 4)
            cast_pool = pool("hcast", 2)
            tl_pool = pool("tload", 2)
            tstage_pool = pool("tstage", 2)
            gh_pool = pool("ghead", 2)
            headst_pool = pool("headst", 1)
            psum_big = pool("psA", 4, space="PSUM")
            psum_T = pool("psT", 2, space="PSUM")
            psum_head = pool("psH", 2, space="PSUM")
            hA = dram.tile([128, KC, BC], F32, tag="hA")
            hB = dram.tile([128, KC, BC], F32, tag="hB")
            g1d = dram.tile([128, KC, BC], BF16, tag="g1d")
            g2d = dram.tile([128, KC, BC], BF16, tag="g2d")

            w2_sb = singles.tile([128, KC, 6], BF16, tag="w2sb")
            nc.sync.dma_start(out=w2_sb, in_=w2_in[:])
            biases = singles.tile([128, NFO], F32, tag="biases")
            nc.sync.dma_start(out=biases, in_=biases_in[:])
            berf = singles.tile([128, 2 * KC], F32, tag="berf")
            nc.sync.dma_start(out=berf, in_=berf_in[:])
            b2 = singles.tile([128, 2], F32, tag="b2")
            nc.sync.dma_start(out=b2, in_=b2_in[:])
            ident = singles.tile([128, 128], F32, tag="ident")
            make_identity(nc, ident[:])

            def transpose_phase(src, dst):
                # src: [128, KC, BC] f32 feature-major -> dst [BC//128, 128, H] batch-major
                for chunk in range(BC // 128):
                    tl = tl_pool.tile([128, KC, 128], F32, tag="tl")
                    nc.sync.dma_start(out=tl, in_=src[:, :, chunk * 128:(chunk + 1) * 128])
                    st = tstage_pool.tile([128, H], F32, tag="st")
                    for half in range(KC // 4):
                        pT = psum_T.tile([128, 4, 128], F32, tag="pT")
                        for jj in range(4):
                            j = half * 4 + jj
                            nc.tensor.transpose(out=pT[:, jj, :], in_=tl[:, j, :], identity=ident[:])
                        nc.vector.tensor_copy(out=st[:, half * 512:(half + 1) * 512], in_=pT[:])
                    nc.sync.dma_start(out=dst[chunk], in_=st)

            for _rep in range(n_repeat):
              for t in range(S):
                rb = h0 if t == 0 else (hA if t % 2 == 1 else hB)
                wb = hA if t % 2 == 0 else hB

                # bf16 cast of this step's h: plain fp32 load + DVE cast
                # (SWDGE dtype-cast DMA is pathologically slow here)
                h_bf = hbf_pool.tile([128, KC, BC], BF16, tag="hbf")
                for b in range(NB):
                    for kh in range(2):
                        ks = slice(kh * KC // 2, (kh + 1) * KC // 2)
                        hc = cast_pool.tile([128, KC // 2, BS], F32, tag="hc")
                        nc.sync.dma_start(out=hc, in_=rb[:, ks, bsl(b)])
                        nc.vector.tensor_copy(out=h_bf[:, ks, bsl(b)], in_=hc[:])

                r_by_b = {}
                z_by_b = {}
                t_by_b = {}
                for j in range(KC):
                    for kind in range(6):  # r, z, hn, in, op1, num1
                        fo = j * 6 + kind
                        w_t = w_pool.tile([128, KC, 128], BF16, tag="wt")
                        nc.sync.dma_start(out=w_t, in_=wbig[:, fo, :])
                        for b in range(NB):
                            ps = psum_big.tile([128, BS], F32, tag="ps")
                            for k in range(KC):
                                nc.tensor.matmul(
                                    ps[:], lhsT=w_t[:, k, :], rhs=h_bf[:, k, bsl(b)],
                                    start=(k == 0), stop=(k == KC - 1),
                                )
                            bcol = biases[:, fo:fo + 1]
                            if kind == 0:
                                r_ = r_pool.tile([128, BS], F32, tag="r")
                                nc.scalar.activation(out=r_[:], in_=ps[:], func=AF.Sigmoid, bias=bcol, scale=1.0)
                                r_by_b[b] = r_
                            elif kind == 1:
                                z_ = z_pool.tile([128, BS], F32, tag="z")
                                nc.scalar.activation(out=z_[:], in_=ps[:], func=AF.Sigmoid, bias=bcol, scale=1.0)
                                z_by_b[b] = z_
                            elif kind == 2:
                                t_ = t_pool.tile([128, BS], F32, tag="t")
                                nc.vector.scalar_tensor_tensor(
                                    out=t_[:], in0=ps[:], scalar=bcol, in1=r_by_b[b][:],
                                    op0=ADD, op1=MULT,
                                )
                                t_by_b[b] = t_
                            elif kind == 3:
                                t2 = t2_pool.tile([128, BS], F32, tag="t2")
                                nc.vector.tensor_add(out=t2[:], in0=ps[:], in1=t_by_b[b][:])
                                n_ = n_pool.tile([128, BS], F32, tag="n")
                                nc.scalar.activation(out=n_[:], in_=t2[:], func=AF.Tanh, bias=bcol, scale=1.0)
                                h32 = h32_pool.tile([128, BS], F32, tag="h32")
                                nc.sync.dma_start(out=h32, in_=rb[:, j, bsl(b)])
                                d_ = d_pool.tile([128, BS], F32, tag="d")
                                nc.vector.tensor_sub(out=d_[:], in0=h32[:], in1=n_[:])
                                e2 = e2_pool.tile([128, BS], F32, tag="e2")
                                nc.vector.tensor_mul(out=e2[:], in0=z_by_b[b][:], in1=d_[:])
                                hn_ = hnew_pool.tile([128, BS], F32, tag="hnw")
                                nc.vector.tensor_add(out=hn_[:], in0=e2[:], in1=n_[:])
                                nc.sync.dma_start(out=wb[:, j, bsl(b)], in_=hn_)
                            else:
                                # op1 (kind 4) -> g1d, num1 (kind 5) -> g2d
                                u_ = u_pool.tile([128, BS], F32, tag="u")
                                nc.vector.tensor_scalar_add(out=u_[:], in0=ps[:], scalar1=bcol)
                                e_ = e_pool.tile([128, BS], F32, tag="e")
                                ecol = berf[:, (0 if kind == 4 else KC) + j:(0 if kind == 4 else KC) + j + 1]
                                nc.scalar.activation(out=e_[:], in_=ps[:], func=AF.Erf, bias=ecol, scale=INV_SQRT2)
                                g_ = g_pool.tile([128, BS], BF16, tag="g")
                                # 2*gelu(u) = u*(1+erf(u/sqrt2)); the 0.5 is folded into w2
                                nc.vector.scalar_tensor_tensor(
                                    out=g_[:], in0=e_[:], scalar=1.0, in1=u_[:], op0=ADD, op1=MULT,
                                )
                                gd = g1d if kind == 4 else g2d
                                nc.sync.dma_start(out=gd[:, j, bsl(b)], in_=g_)

                # states[t] = pre-update h = h_{t+1} written this step? No:
                # states[t] is THIS step's input h; host supplies states[0]=x,
                # so transpose rb (t>=1) into states_o[t-1].
                if t >= 1:
                    transpose_phase(rb, states_o[t - 1])

                # head matmuls from DRAM-staged gelu activations
                for b in range(NB):
                    gh1 = gh_pool.tile([128, KC, BS], BF16, tag="gh")
                    nc.sync.dma_start(out=gh1, in_=g1d[:, :, bsl(b)])
                    po = psum_head.tile([5, BS], F32, tag="ph")
                    for k in range(KC):
                        nc.tensor.matmul(po[:], lhsT=w2_sb[:, k, 0:5], rhs=gh1[:, k, :],
                                         start=(k == 0), stop=(k == KC - 1))
                    so = headst_pool.tile([5, BS], F32, tag="hso")
                    nc.vector.tensor_scalar_add(out=so[:], in0=po[:], scalar1=b2[0:5, 0:1])
                    nc.sync.dma_start(out=ops_o[t, :, bsl(b)], in_=so)

                    gh2 = gh_pool.tile([128, KC, BS], BF16, tag="gh")
                    nc.sync.dma_start(out=gh2, in_=g2d[:, :, bsl(b)])
                    pn = psum_head.tile([1, BS], F32, tag="ph")
                    for k in range(KC):
                        nc.tensor.matmul(pn[:], lhsT=w2_sb[:, k, 5:6], rhs=gh2[:, k, :],
                                         start=(k == 0), stop=(k == KC - 1))
                    sn = headst_pool.tile([1, BS], F32, tag="hsn")
                    nc.vector.tensor_scalar_add(out=sn[:], in0=pn[:], scalar1=b2[0:1, 1:2])
                    nc.sync.dma_start(out=nums_o[t, :, bsl(b)], in_=sn)

            # final_state = h_{S} = wb of last step
            final_src = hA if (S - 1) % 2 == 0 else hB
            transpose_phase(final_src, final_o)

    _split_excess_waits(nc, max_per=1)
    return nc


_NC_CACHE = {}


def _get_nc(num_steps):
    if num_steps not in _NC_CACHE:
        _NC_CACHE[num_steps] = build_nc(num_steps)
    return _NC_CACHE[num_steps]


def _prep_weights(w_op1, b_op1, w_op2, b_op2, w_num1, b_num1, w_num2, b_num2,
                  w_ih, b_ih, w_hh, b_hh):
    f = np.float32
    w_ih = np.asarray(w_ih, f); w_hh = np.asarray(w_hh, f)
    b_ih = np.asarray(b_ih, f); b_hh = np.asarray(b_hh, f)
    Wr = (w_ih[0:H] + w_hh[0:H]).T.astype(f)          # [H, H]
    Wz = (w_ih[H:2 * H] + w_hh[H:2 * H]).T.astype(f)
    Whn = w_hh[2 * H:].T.astype(f)
    Win = w_ih[2 * H:].T.astype(f)
    br = (b_ih[0:H] + b_hh[0:H]).astype(f)
    bz = (b_ih[H:2 * H] + b_hh[H:2 * H]).astype(f)
    bhn = b_hh[2 * H:].astype(f)
    bin_ = b_ih[2 * H:].astype(f)
    w_op1 = np.asarray(w_op1, f); w_num1 = np.asarray(w_num1, f)
    b_op1 = np.asarray(b_op1, f); b_num1 = np.asarray(b_num1, f)

    # fused weight: per j-group of 128 output features: [r, z, hn, in, op1, num1]
    Wcat = np.empty((H, NFO * 128), f)
    bias_cols = np.empty((128, NFO), f)
    kinds = [(Wr, br), (Wz, bz), (Whn, bhn), (Win, bin_), (w_op1, b_op1), (w_num1, b_num1)]
    for j in range(KC):
        for kind, (Wk, bk) in enumerate(kinds):
            fo = j * 6 + kind
            Wcat[:, fo * 128:(fo + 1) * 128] = Wk[:, j * 128:(j + 1) * 128]
            bias_cols[:, fo] = bk[j * 128:(j + 1) * 128]

    # wbig[p, fo, k*128+m] = Wcat[k*128+p, fo*128+m]
    wbig = np.ascontiguousarray(
        Wcat.reshape(KC, 128, NFO, 128).transpose(1, 2, 0, 3).reshape(128, NFO, H)
    ).astype(ml_dtypes.bfloat16)

    berf = np.empty((128, 2 * KC), f)
    for j in range(KC):
        berf[:, j] = b_op1[j * 128:(j + 1) * 128] * INV_SQRT2
        berf[:, KC + j] = b_num1[j * 128:(j + 1) * 128] * INV_SQRT2

    w2 = np.zeros((128, KC, 6), f)
    w2[:, :, 0:5] = 0.5 * np.asarray(w_op2, f).reshape(KC, 128, 5).transpose(1, 0, 2)
    w2[:, :, 5] = 0.5 * np.asarray(w_num2, f).reshape(KC, 128).transpose(1, 0)
    w2 = w2.astype(ml_dtypes.bfloat16)

    b2 = np.zeros((128, 2), f)
    b2[0:5, 0] = np.asarray(b_op2, f)
    b2[0:1, 1] = np.asarray(b_num2, f)
    return wbig, bias_cols, berf, w2, b2


def kernel(x, num_steps, w_op1, b_op1, w_op2, b_op2,
           w_num1, b_num1, w_num2, b_num2,
           w_ih, b_ih, w_hh, b_hh):
    S = int(num_steps)
    x = np.asarray(x, np.float32)
    assert x.shape == (B_FULL, H)

    wbig, bias_cols, berf, w2, b2 = _prep_weights(
        w_op1, b_op1, w_op2, b_op2, w_num1, b_num1, w_num2, b_num2,
        w_ih, b_ih, w_hh, b_hh)

    nc = _get_nc(S)
    in_maps = []
    for c in range(N_CORES):
        xs = x[c * BC:(c + 1) * BC]                       # [BC, H]
        h0 = np.ascontiguousarray(
            xs.T.reshape(KC, 128, BC).transpose(1, 0, 2))  # [128, KC, BC]
        in_maps.append({
            "h0": h0, "wbig": wbig, "w2": w2,
            "biases": bias_cols, "berf": berf, "b2": b2,
        })

    res = run_bass_kernel_spmd(nc, in_maps, list(range(N_CORES)))
    results = res.results

    final = np.concatenate([r["final_o"].reshape(BC, H) for r in results], axis=0)
    states = np.empty((S, B_FULL, H), np.float32)
    states[0] = x
    for t in range(1, S):
        states[t] = np.concatenate(
            [r["states_o"][t - 1].reshape(BC, H) for r in results], axis=0)
    ops = np.concatenate([r["ops_o"].transpose(0, 2, 1) for r in results], axis=1)
    nums = np.concatenate([r["nums_o"].transpose(0, 2, 1) for r in results], axis=1)
    return final, ops, nums, states
